# revision 1
# baseline (speedup 1.0000x reference)
"""Luong seq2seq (2-layer BiGRU encoder + attention GRU decoder + vocab
projection) as a single 8-core SPMD Bass/Tile kernel for Trainium2.

Sharding: data-parallel over batch (64 examples -> 8 per core). Each core
runs the full recurrence for its 8 examples and projects onto the full
32000-word vocabulary; the host concatenates per-core logits.

Self-contained: hardcodes all shapes; takes the full unsharded inputs of
reference.setup_inputs() and returns the full (48, 64, 32000) logits.
"""

import os
import sys
import types

for _p in ("/opt/trn_rl_repo", "/opt/pypackages", "/root/.axon_site",
           "/root/.axon_site/_ro/trn_rl_repo", "/root/.axon_site/_ro/pypackages"):
    if os.path.isdir(_p) and _p not in sys.path:
        sys.path.append(_p)

import numpy as np

from concourse import bass, mybir, tile, bacc
from concourse import bass_utils
from concourse.bass_utils import run_bass_kernel_spmd
from concourse.masks import make_identity

# ---------------------------------------------------------------- constants
V, H, T, B, NCORES = 32000, 512, 48, 64, 8
Bc = B // NCORES            # 8 examples per core
H2, H3 = 2 * H, 3 * H
NSEQ = T * Bc               # 384 (t-major row order: r = t*Bc + b)
NSCAN = T * 2 * Bc          # 768 (enc scan rows: r = t*16 + lane*8 + b)
P = 128
NEG = -1.0e9

f32 = mybir.dt.float32
f32r = mybir.dt.float32r
bf16 = mybir.dt.bfloat16
i32 = mybir.dt.int32
AF = mybir.ActivationFunctionType
OP = mybir.AluOpType

VCHUNKS = [(i * 512, 512) for i in range(62)] + [(62 * 512, 256)]  # 32000


def _install_profile_hook():
    """Make trace=True work: the image's antenv lacks axon_hooks."""
    if "antenv.axon_hooks" in sys.modules:
        return
    try:
        import trn_agent_boot.trn_boot as tb
        hook = tb._ntff_profile_via_ctypes("/opt/axon/libaxon_pjrt.so")
        m = types.ModuleType("antenv.axon_hooks")
        m.get_axon_ntff_profile_hook = lambda: hook
        m.set_axon_ntff_profile_hook = lambda h: None
        sys.modules["antenv.axon_hooks"] = m
        import antenv
        antenv.axon_hooks = m
        bass_utils.upload_artifacts = lambda d: d
    except Exception:
        pass


# ---------------------------------------------------------------- program
def build_program(dbg=False):
    nc = bacc.Bacc("TRN2", target_bir_lowering=False, debug=False,
                   num_devices=NCORES)

    def din(name, shape, dt=f32r):
        return nc.dram_tensor(name, list(shape), dt, kind="ExternalInput").ap()

    io = {}
    io["xeT_in"] = din("xeT_in", (H, NSCAN))
    io["xdT_in"] = din("xdT_in", (H, NSEQ))
    io["amask"] = din("amask", (Bc, NSEQ), f32)
    for name, shape in [
        ("w0t", (H, H3)), ("u0", (H, H3)), ("b0", (1, H3)), ("bn0", (1, H)),
        ("w1t", (H2, H3)), ("u1", (H, H3)), ("b1", (1, H3)), ("bn1", (1, H)),
        ("fct", (H2, H)), ("fcb", (1, H)), ("was", (H2, H)),
        ("wcc", (H2, H)), ("wch", (H, H)),
        ("wxd", (H, H3)), ("whd", (H, H3)), ("bd", (1, H3)),
        ("ud", (H, H3)), ("bnd", (1, H)),
    ]:
        io[name] = din(name, shape)
    io["owt"] = din("owt", (H, V), bf16)
    io["out"] = nc.dram_tensor("out", [NSEQ, V], f32, kind="ExternalOutput").ap()
    io["dbg"] = dbg
    if dbg:
        io["dbg_xp0"] = nc.dram_tensor("dbg_xp0", [NSCAN, H3], f32r,
                                       kind="ExternalOutput").ap()
        io["dbg_l0"] = nc.dram_tensor("dbg_l0", [NSCAN, H2], f32,
                                      kind="ExternalOutput").ap()
        io["dbg_henc"] = nc.dram_tensor("dbg_henc", [NSEQ, H2], f32,
                                        kind="ExternalOutput").ap()
        io["dbg_h0"] = nc.dram_tensor("dbg_h0", [Bc, H], f32,
                                      kind="ExternalOutput").ap()
        io["dbg_htall"] = nc.dram_tensor("dbg_htall", [P, 4 * NSEQ], f32,
                                         kind="ExternalOutput").ap()
        io["dbg_xpx"] = nc.dram_tensor("dbg_xpx", [NSEQ, H3], f32r,
                                       kind="ExternalOutput").ap()
        io["dbg_hall"] = nc.dram_tensor("dbg_hall", [NSEQ, H], f32,
                                        kind="ExternalOutput").ap()


    with tile.TileContext(nc) as tc:
        _emit(nc, tc, io)
    nc.compile()
    return nc


def _emit(nc, tc, io):
    # ------- long-lived pools
    cpool_cm = tc.tile_pool(name="const", bufs=1)
    spool_cm = tc.tile_pool(name="state", bufs=2)
    wpool_cm = tc.tile_pool(name="work", bufs=2)
    wpool4_cm = tc.tile_pool(name="work4", bufs=4)
    xpool_cm = tc.tile_pool(name="xstage", bufs=3)
    dpool_cm = tc.tile_pool(name="dram", bufs=1, space="DRAM")
    pp_cm = tc.tile_pool(name="pp", bufs=1, space="PSUM")
    pt_cm = tc.tile_pool(name="pt", bufs=2, space="PSUM")
    ps_cm = tc.tile_pool(name="ps", bufs=2, space="PSUM")
    cpool = cpool_cm.__enter__()
    spool = spool_cm.__enter__()
    wpool = wpool_cm.__enter__()
    wpool4 = wpool4_cm.__enter__()
    xpool = xpool_cm.__enter__()
    dpool = dpool_cm.__enter__()
    pp = pp_cm.__enter__()
    pt = pt_cm.__enter__()
    ps = ps_cm.__enter__()

    # ---------------- constants
    ident = cpool.tile([P, P], f32)
    make_identity(nc, ident[:])
    identr = cpool.tile([P, P], f32r)
    nc.vector.tensor_copy(identr[:], ident[:])
    ones_f = cpool.tile([1, P], f32)
    nc.vector.memset(ones_f[:], 1.0)
    ones = cpool.tile([1, P], f32r)
    nc.vector.tensor_copy(ones[:], ones_f[:])

    def load_const(name, shape):
        t = cpool.tile(list(shape), f32r, tag=name)
        nc.sync.dma_start(t[:], io[name][:])
        return t

    bn0_sb = load_const("bn0", (1, H))
    bn1_sb = load_const("bn1", (1, H))
    bnd_sb = load_const("bnd", (1, H))
    fcb_sb = load_const("fcb", (1, H))
    amask_sb = cpool.tile([Bc, NSEQ], f32, tag="amask")
    nc.sync.dma_start(amask_sb[:], io["amask"][:])

    # DRAM scratch
    xp0_d = dpool.tile([NSCAN, H3], f32r)
    xp1_d = dpool.tile([NSCAN, H3], f32r)
    xpx_d = dpool.tile([NSEQ, H3], f32r)
    l0_d = dpool.tile([NSCAN, H2], f32)
    henc_d = dpool.tile([NSEQ, H2], f32)

    # ---------------- helpers
    def kload(pool, name, kdim, n, tag):
        ko = kdim // P
        t = pool.tile([P, ko, n], f32r, tag=tag)
        nc.sync.dma_start(t[:], io[name].rearrange("(ko p) n -> p ko n", p=P))
        return t

    def batched_mm(out_dram, lhsT_tile, kdim, mtiles, rhs_name, nbase,
                   bias_sb, opool):
        """out[m*128.., :nbase] = lhsT.T @ io[rhs_name] + bias -> DRAM f32r.

        Streams the rhs weight in (128, ko, 512) column chunks."""
        ko = kdim // P
        rhs_r = io[rhs_name].rearrange("(ko p) n -> p ko n", p=P)
        for c0 in range(0, nbase, 512):
            cw = min(512, nbase - c0)
            rhs_c = opool.tile([P, ko, 512], f32r, tag="rhsc")
            nc.sync.dma_start(rhs_c[:, :, :cw], rhs_r[:, :, c0:c0 + cw])
            for m in range(mtiles):
                ps_t = ps.tile([P, 512], f32, tag="sc")
                for k in range(ko):
                    nc.tensor.matmul(ps_t[:, :cw],
                                     lhsT_tile[:, k, m * P:(m + 1) * P],
                                     rhs_c[:, k, :cw],
                                     start=(k == 0), stop=False)
                nc.tensor.matmul(ps_t[:, :cw], ones[:1, :P],
                                 bias_sb[:1, c0:c0 + cw],
                                 start=False, stop=True)
                ob = opool.tile([P, 512], f32r, tag="mmob")
                nc.scalar.copy(out=ob[:, :cw], in_=ps_t[:, :cw])
                nc.sync.dma_start(out_dram[m * P:(m + 1) * P, c0:c0 + cw],
                                  ob[:, :cw])

    def transpose_to(dst_ap, src_ap, rows, ident_t, eng):
        """dst_ap (128, rows) = src_ap (rows, 128) transposed."""
        tp_t = pt.tile([P, P], src_ap.dtype, tag="tp")
        nc.tensor.transpose(tp_t[:, :rows], src_ap, ident_t[:rows, :rows])
        eng.copy(out=dst_ap, in_=tp_t[:, :rows])

    # =========================================================== gather phase
    gpool_cm = tc.tile_pool(name="gather", bufs=1)
    gwork_cm = tc.tile_pool(name="gwork", bufs=2)
    gpool = gpool_cm.__enter__()
    gwork = gwork_cm.__enter__()

    xeT = kload(gpool, "xeT_in", H, NSCAN, "xeT")
    xdT = kload(gpool, "xdT_in", H, NSEQ, "xdT")

    b0_sb = gpool.tile([1, H3], f32r, tag="b0")
    nc.sync.dma_start(b0_sb[:], io["b0"][:])
    bd_sb2 = gpool.tile([1, H3], f32r, tag="bd")
    nc.sync.dma_start(bd_sb2[:], io["bd"][:])
    batched_mm(xp0_d[:], xeT, H, 6, "w0t", H3, b0_sb, gwork)
    batched_mm(xpx_d[:], xdT, H, 3, "wxd", H3, bd_sb2, gwork)

    gwork_cm.__exit__(None, None, None)
    gpool_cm.__exit__(None, None, None)

    # =========================================================== GRU scan
    def gru_scan(nsteps, m_rows, xp_dram, u_sb, whx_sb, bn_sb, h0_sb, h0T,
                 step_out):
        h_sb, hT = h0_sb, h0T
        for t in range(nsteps):
            whT = step_out.ht_T if whx_sb is not None else None
            xp_t = xpool.tile([m_rows, H3], f32r, tag="xpt")
            nc.sync.dma_start(xp_t[:], xp_dram[t * m_rows:(t + 1) * m_rows, :])
            # regions: 0/1 = r/z pre-acts (xp + hU [+ htWh]);
            # 2 = bn + hU_n (inside r*(.)); 3 (dec only) = xp_n + htWh_n
            nreg = 4 if whx_sb is not None else 3
            p = pp.tile([m_rows, 4 * 512], f32, tag="p")
            for c in range(nreg):
                c0 = c * 512
                seg = p[:, c0:c0 + 512]
                mms = []
                if c < 2:
                    mms.append((identr[:m_rows, :m_rows], xp_t[:, c0:c0 + 512]))
                elif c == 2:
                    mms.append((ones[:1, :m_rows], bn_sb[:1, :]))
                else:
                    mms.append((identr[:m_rows, :m_rows], xp_t[:, H2:H3]))
                if hT is not None and c < 3:
                    for k in range(4):
                        mms.append((hT[:, k, :m_rows], u_sb[:, k, c0:c0 + 512]))
                if whT is not None and c != 2:
                    wc0 = c0 if c < 2 else H2
                    for k in range(4):
                        mms.append((whT[:, k, :m_rows],
                                    whx_sb[:, k, wc0:wc0 + 512]))
                last = len(mms) - 1
                for i, (lh, rh) in enumerate(mms):
                    nc.tensor.matmul(seg, lh, rh, start=(i == 0),
                                     stop=(i == last))

            def warm():
                # dummy matmul to keep the PE HAM clock at 2.4 GHz through
                # the gate-chain idle window (results never read)
                w = ps.tile([m_rows, 512], f32, tag="sc")
                nc.tensor.matmul(w[:, :], identr[:m_rows, :m_rows],
                                 xp_t[:, 0:512], start=True, stop=True)

            n_warm = 3 if whx_sb is None else (3 if t < 16 else 0)
            if n_warm:
                warm()
            # gates (r-sigmoid first so DVE work overlaps the z-sigmoid)
            rz = wpool.tile([m_rows, H2], f32, tag="rz")
            nc.scalar.activation(rz[:, 0:H], p[:, 0:H], AF.Sigmoid)
            rn = wpool4.tile([m_rows, H], f32, tag="g1")
            nc.vector.tensor_mul(rn[:], rz[:, 0:H], p[:, H2:H3])
            nin = wpool4.tile([m_rows, H], f32, tag="g1")
            if whx_sb is not None:
                nc.vector.tensor_add(nin[:], rn[:], p[:, 3 * 512:4 * 512])
            else:
                nc.vector.tensor_add(nin[:], rn[:], xp_t[:, H2:H3])
            nc.scalar.activation(rz[:, H:H2], p[:, H:H2], AF.Sigmoid)
            if n_warm > 1:
                warm()
            n_t = wpool4.tile([m_rows, H], f32, tag="g1")
            nc.scalar.activation(n_t[:], nin[:], AF.Tanh)
            if n_warm > 2:
                warm()
            omz = wpool4.tile([m_rows, H], f32, tag="g1")
            nc.vector.tensor_scalar(omz[:], rz[:, H:H2], -1.0, 1.0,
                                    OP.mult, OP.add)
            h_new = spool.tile([m_rows, H], f32, tag="h")
            if h_sb is not None:
                zh = wpool4.tile([m_rows, H], f32, tag="g1")
                nc.gpsimd.tensor_mul(zh[:], rz[:, H:H2], h_sb[:])
                hn1 = wpool4.tile([m_rows, H], f32, tag="g1")
                nc.vector.tensor_mul(hn1[:], omz[:], n_t[:])
                nc.vector.tensor_add(h_new[:], hn1[:], zh[:])
            else:
                nc.vector.tensor_mul(h_new[:], omz[:], n_t[:])
            hT_new = spool.tile([P, 4, m_rows], f32r, tag="hT")
            for k in range(4):
                transpose_to(hT_new[:, k, :], h_new[:, k * P:(k + 1) * P],
                             m_rows, ident, nc.scalar)
            step_out.emit(t, h_new, hT_new)
            h_sb, hT = h_new, hT_new
        return h_sb, hT

    class EncOut:
        ht_T = None

        def __init__(self, kind):
            self.kind = kind

        def emit(self, s, h_new, hT_new):
            if self.kind == "l0":
                d = l0_d
                nc.sync.dma_start(d[s * 16:s * 16 + 8, 0:H], h_new[0:8, :])
                nc.sync.dma_start(
                    d[(T - 1 - s) * 16 + 8:(T - 1 - s) * 16 + 16, 0:H],
                    h_new[0:8, :])
                nc.sync.dma_start(
                    d[(T - 1 - s) * 16:(T - 1 - s) * 16 + 8, H:H2],
                    h_new[8:16, :])
                nc.sync.dma_start(d[s * 16 + 8:s * 16 + 16, H:H2],
                                  h_new[8:16, :])
            else:
                d = henc_d
                nc.sync.dma_start(d[s * Bc:(s + 1) * Bc, 0:H], h_new[0:8, :])
                nc.sync.dma_start(d[(T - 1 - s) * Bc:(T - s) * Bc, H:H2],
                                  h_new[8:16, :])

    # ---- encoder
    e0pool_cm = tc.tile_pool(name="encp", bufs=1)
    e0work_cm = tc.tile_pool(name="encw", bufs=2)
    e0pool = e0pool_cm.__enter__()
    e0work = e0work_cm.__enter__()

    u0_sb = kload(e0pool, "u0", H, H3, "u0sb")
    gru_scan(T, 16, xp0_d[:], u0_sb, None, bn0_sb, None, None, EncOut("l0"))

    l0T = e0pool.tile([P, 8, NSCAN], f32r, tag="l0T")
    for m in range(6):
        lrow = e0work.tile([P, H2], f32, tag="lrow")
        nc.sync.dma_start(lrow[:], l0_d[m * P:(m + 1) * P, :])
        for k in range(8):
            transpose_to(l0T[:, k, m * P:(m + 1) * P],
                         lrow[:, k * P:(k + 1) * P], P, ident, nc.scalar)
    b1_sb = e0pool.tile([1, H3], f32r, tag="b1")
    nc.sync.dma_start(b1_sb[:], io["b1"][:])
    batched_mm(xp1_d[:], l0T, H2, 6, "w1t", H3, b1_sb, e0work)

    u1_sb = kload(e0pool, "u1", H, H3, "u0sb")   # reuse u0 slot
    gru_scan(T, 16, xp1_d[:], u1_sb, None, bn1_sb, None, None, EncOut("henc"))

    e0work_cm.__exit__(None, None, None)
    e0pool_cm.__exit__(None, None, None)

    # =========================================================== attention pre
    mpool_cm = tc.tile_pool(name="mid", bufs=1)
    mwork_cm = tc.tile_pool(name="midw", bufs=2)
    mpool = mpool_cm.__enter__()
    mwork = mwork_cm.__enter__()

    hencT = mpool.tile([P, 8, NSEQ], f32r, tag="hencT")
    for m in range(3):
        hrow = mwork.tile([P, H2], f32, tag="hrow")
        nc.sync.dma_start(hrow[:], henc_d[m * P:(m + 1) * P, :])
        for k in range(8):
            transpose_to(hencT[:, k, m * P:(m + 1) * P],
                         hrow[:, k * P:(k + 1) * P], P, ident, nc.scalar)

    was_sb = kload(mwork, "was", H2, H, "wpre")
    gT = mpool.tile([P, 4, NSEQ], f32r, tag="gT")
    for m in range(4):
        ps_t = ps.tile([P, 512], f32, tag="sc")
        for k in range(8):
            nc.tensor.matmul(ps_t[:, :NSEQ], was_sb[:, k, m * P:(m + 1) * P],
                             hencT[:, k, :], start=(k == 0), stop=(k == 7))
        nc.scalar.copy(out=gT[:, m, :], in_=ps_t[:, :NSEQ])

    wcc_sb = kload(mwork, "wcc", H2, H, "wpre")
    pf = mpool.tile([P, 3, H], f32r, tag="pf")
    for m in range(3):
        ps_t = ps.tile([P, 512], f32, tag="sc")
        for k in range(8):
            nc.tensor.matmul(ps_t[:, :H], hencT[:, k, m * P:(m + 1) * P],
                             wcc_sb[:, k, :], start=(k == 0), stop=(k == 7))
        nc.scalar.copy(out=pf[:, m, :], in_=ps_t[:, :H])

    fct_sb = kload(mwork, "fct", H2, H, "wpre")
    h0p = ps.tile([P, 512], f32, tag="sc")
    for k in range(8):
        c0 = (T - 1) * Bc if k < 4 else 0
        nc.tensor.matmul(h0p[:Bc, :H], hencT[:, k, c0:c0 + Bc],
                         fct_sb[:, k, :], start=(k == 0), stop=False)
    nc.tensor.matmul(h0p[:Bc, :H], ones[:1, :Bc], fcb_sb[:1, :],
                     start=False, stop=True)
    h0_sb = spool.tile([Bc, H], f32, tag="h")
    nc.scalar.activation(h0_sb[:], h0p[:Bc, :H], AF.Tanh)
    h0T = spool.tile([P, 4, Bc], f32r, tag="hT")
    for k in range(4):
        transpose_to(h0T[:, k, :], h0_sb[:, k * P:(k + 1) * P], Bc, ident,
                     nc.scalar)

    # =========================================================== decoder
    ud_sb = kload(mpool, "ud", H, H3, "udsb")
    whd_sb = kload(mpool, "whd", H, H3, "whdsb")
    wch_sb = kload(mpool, "wch", H, H, "wchsb")
    htall = mpool.tile([P, 4, NSEQ], bf16, tag="htall")

    class DecOut:
        ht_T = None

        def emit(self, t, h_new, hT_new):
            if io["dbg"]:
                nc.sync.dma_start(io["dbg_hall"][t * Bc:(t + 1) * Bc, :],
                                  h_new[:, :])
            sc_ps = ps.tile([P, 512], f32, tag="sc")
            for k in range(4):
                nc.tensor.matmul(sc_ps[:Bc, :NSEQ], hT_new[:, k, :],
                                 gT[:, k, :], start=(k == 0), stop=(k == 3))
            sc = wpool.tile([Bc, NSEQ], f32, tag="scb")
            nc.vector.tensor_add(sc[:], sc_ps[:Bc, :NSEQ], amask_sb[:])
            alpha = wpool.tile([Bc, NSEQ], f32, tag="alpha")
            sexp = wpool.tile([Bc, 1], f32, tag="sexp")
            nc.scalar.activation(alpha[:], sc[:], AF.Exp, accum_out=sexp[:])
            rs = wpool.tile([Bc, 1], f32, tag="rs")
            nc.vector.reciprocal(rs[:], sexp[:])
            nc.vector.tensor_scalar_mul(alpha[:], alpha[:], rs[:])
            aT = wpool.tile([P, 3, Bc], f32r, tag="aT")
            for j in range(3):
                transpose_to(aT[:, j, :], alpha[:, j * P:(j + 1) * P], Bc,
                             ident, nc.scalar)
            htp = ps.tile([P, 512], f32, tag="sc")
            for j in range(3):
                nc.tensor.matmul(htp[:Bc, :H], aT[:, j, :], pf[:, j, :],
                                 start=(j == 0), stop=False)
            for k in range(4):
                nc.tensor.matmul(htp[:Bc, :H], hT_new[:, k, :],
                                 wch_sb[:, k, :], start=False, stop=(k == 3))
            ht_sb = wpool4.tile([Bc, H], f32, tag="g1")
            nc.scalar.activation(ht_sb[:], htp[:Bc, :H], AF.Tanh)
            ht_T = spool.tile([P, 4, Bc], f32r, tag="htT")
            for k in range(4):
                tp_t = pt.tile([P, P], f32, tag="tp")
                nc.tensor.transpose(tp_t[:, :Bc], ht_sb[:, k * P:(k + 1) * P],
                                    ident[:Bc, :Bc])
                nc.scalar.copy(out=ht_T[:, k, :], in_=tp_t[:, :Bc])
                nc.vector.tensor_copy(out=htall[:, k, t * Bc:(t + 1) * Bc],
                                      in_=tp_t[:, :Bc])
            self.ht_T = ht_T

    if io["dbg"]:
        nc.sync.dma_start(io["dbg_h0"][:], h0_sb[:])

    mwork_cm.__exit__(None, None, None)

    # ============== decoder with interleaved bf16 vocab projection
    prpool_cm = tc.tile_pool(name="proj", bufs=3)
    prpool = prpool_cm.__enter__()
    owt_r = io["owt"].rearrange("(ko p) v -> p ko v", p=P)

    def emit_proj(m, c0, cw):
        ow = prpool.tile([P, 4, 512], bf16, tag="ow")
        nc.sync.dma_start(ow[:, :, :cw], owt_r[:, :, c0:c0 + cw])
        ps_t = ps.tile([P, 512], f32, tag="sc")
        for k in range(4):
            nc.tensor.matmul(ps_t[:, :cw], htall[:, k, m * P:(m + 1) * P],
                             ow[:, k, :cw], start=(k == 0), stop=(k == 3))
        ob = prpool.tile([P, 512], f32, tag="ob")
        nc.scalar.copy(out=ob[:, :cw], in_=ps_t[:, :cw])
        nc.sync.dma_start(io["out"][m * P:(m + 1) * P, c0:c0 + cw],
                          ob[:, :cw])

    todo = {m: list(VCHUNKS) for m in range(3)}

    class DecOutP(DecOut):
        def emit(self, t, h_new, hT_new):
            super().emit(t, h_new, hT_new)
            if t >= 16:
                m = (t - 16) // 16
                for _ in range(5):
                    if todo[m]:
                        c0, cw = todo[m].pop(0)
                        emit_proj(m, c0, cw)

    dec_out = DecOutP()
    gru_scan(T, Bc, xpx_d[:], ud_sb, whd_sb, bnd_sb, h0_sb, h0T, dec_out)
    if io["dbg"]:
        nc.sync.dma_start(io["dbg_xp0"][:], xp0_d[:])
        nc.sync.dma_start(io["dbg_l0"][:], l0_d[:])
        nc.sync.dma_start(io["dbg_henc"][:], henc_d[:])
        nc.sync.dma_start(io["dbg_xpx"][:], xpx_d[:])
    for m in range(3):
        while todo[m]:
            c0, cw = todo[m].pop(0)
            emit_proj(m, c0, cw)
    prpool_cm.__exit__(None, None, None)
    mpool_cm.__exit__(None, None, None)

    for cm in (ps_cm, pt_cm, pp_cm, dpool_cm, xpool_cm, wpool4_cm, wpool_cm,
               spool_cm, cpool_cm):
        cm.__exit__(None, None, None)


# ---------------------------------------------------------------- host side
_PROGRAM = None


def _get_program():
    global _PROGRAM
    if _PROGRAM is None:
        _install_profile_hook()
        _PROGRAM = build_program()
    return _PROGRAM


def _prep_shared(inputs):
    f = np.float32
    g = {}
    g["w0t"] = np.ascontiguousarray(np.asarray(inputs["enc0_Wih"], f).T)
    g["u0"] = np.concatenate([np.asarray(inputs["enc0_Ur"], f).T,
                              np.asarray(inputs["enc0_Uz"], f).T,
                              np.asarray(inputs["enc0_Un"], f).T], axis=1)
    g["b0"] = np.asarray(inputs["enc0_bih"], f)[None, :]
    g["bn0"] = np.asarray(inputs["enc0_bn"], f)[None, :]
    g["w1t"] = np.ascontiguousarray(np.asarray(inputs["enc1_Wih"], f).T)
    g["u1"] = np.concatenate([np.asarray(inputs["enc1_Ur"], f).T,
                              np.asarray(inputs["enc1_Uz"], f).T,
                              np.asarray(inputs["enc1_Un"], f).T], axis=1)
    g["b1"] = np.asarray(inputs["enc1_bih"], f)[None, :]
    g["bn1"] = np.asarray(inputs["enc1_bn"], f)[None, :]
    g["fct"] = np.ascontiguousarray(np.asarray(inputs["fc_init_w"], f).T)
    g["fcb"] = np.asarray(inputs["fc_init_b"], f)[None, :]
    scale = np.float32(1.0) / np.sqrt(np.float32(H2))
    g["was"] = np.ascontiguousarray(np.asarray(inputs["Wa"], f) * scale)
    acw = np.asarray(inputs["attn_combine_w"], f)
    g["wch"] = np.ascontiguousarray(acw[:, :H].T)
    g["wcc"] = np.ascontiguousarray(acw[:, H:].T)
    dwih = np.asarray(inputs["dec_Wih"], f)
    g["wxd"] = np.ascontiguousarray(dwih[:, :H].T)
    g["whd"] = np.ascontiguousarray(dwih[:, H:].T)
    g["bd"] = np.asarray(inputs["dec_bih"], f)[None, :]
    g["ud"] = np.concatenate([np.asarray(inputs["dec_Ur"], f).T,
                              np.asarray(inputs["dec_Uz"], f).T,
                              np.asarray(inputs["dec_Un"], f).T], axis=1)
    g["bnd"] = np.asarray(inputs["dec_bn"], f)[None, :]
    import ml_dtypes
    g["owt"] = np.ascontiguousarray(
        np.asarray(inputs["out_w"], f).T.astype(ml_dtypes.bfloat16))
    for k in g:
        g[k] = np.ascontiguousarray(g[k])
    return g


def _prep_core(inputs, c):
    src = np.asarray(inputs["src"])
    tgt = np.asarray(inputs["tgt"])
    emb = np.asarray(inputs["emb"], np.float32)
    si = src[:, c * Bc:(c + 1) * Bc].astype(np.int64)      # (48, 8)
    ti = tgt[:, c * Bc:(c + 1) * Bc].astype(np.int64)
    idx_enc = np.empty((T, 2, Bc), np.int64)
    idx_enc[:, 0, :] = si
    idx_enc[:, 1, :] = si[::-1]
    xeT_in = np.ascontiguousarray(emb[idx_enc.reshape(NSCAN)].T)
    xdT_in = np.ascontiguousarray(emb[ti.reshape(NSEQ)].T)
    m = np.full((Bc, T, Bc), NEG, np.float32)
    for b in range(Bc):
        m[b, :, b] = np.where(si[:, b] != 0, np.float32(0.0), np.float32(NEG))
    return {"xeT_in": xeT_in,
            "xdT_in": xdT_in,
            "amask": m.reshape(Bc, NSEQ)}


def kernel(**inputs):
    nc = _get_program()
    shared = _prep_shared(inputs)
    in_maps = []
    for c in range(NCORES):
        im = dict(shared)
        im.update(_prep_core(inputs, c))
        in_maps.append(im)
    res = run_bass_kernel_spmd(nc, in_maps, core_ids=list(range(NCORES)))
    logits = np.empty((T, B, V), np.float32)
    for c in range(NCORES):
        logits[:, c * Bc:(c + 1) * Bc, :] = \
            res.results[c]["out"].reshape(T, Bc, V)
    return logits



# revision 15
# speedup vs baseline: 1.1251x; 1.1251x over previous
"""Luong seq2seq (2-layer BiGRU encoder + attention GRU decoder + vocab
projection) as a single 8-core SPMD Bass/Tile kernel for Trainium2.

v2: bf16 matmuls everywhere, transposed-land elementwise tail (ops on
[128, 4*m] instead of [m, 512]), sigmoid-via-tanh (single activation
table, r/z input projections pre-scaled by 0.5 on the host), SBUF-
resident scratch with incremental l0T/hencT build (no per-step DMAs,
no transpose phases), greedy interleaved vocab projection, bf16 logits
output (host casts to f32).

Sharding: data-parallel over batch (64 -> 8 per core); each core
projects onto the full 32000-word vocab.

Self-contained: hardcodes all shapes; takes the full unsharded inputs
of reference.setup_inputs() and returns the full (48, 64, 32000) f32
logits.
"""

import os
import sys
import types

for _p in ("/opt/trn_rl_repo", "/opt/pypackages", "/root/.axon_site",
           "/root/.axon_site/_ro/trn_rl_repo", "/root/.axon_site/_ro/pypackages"):
    if os.path.isdir(_p) and _p not in sys.path:
        sys.path.append(_p)

import numpy as np

from concourse import bass, mybir, tile, bacc
from concourse import bass_utils
from concourse.bass_utils import run_bass_kernel_spmd
from concourse.masks import make_identity

# ---------------------------------------------------------------- constants
V, H, T, B, NCORES = 32000, 512, 48, 64, 8
Bc = B // NCORES            # 8 examples per core
H2, H3 = 2 * H, 3 * H
NSEQ = T * Bc               # 384 decoder rows (t-major: r = t*Bc + b)
NSCAN = T * 2 * Bc          # 768 encoder scan rows (r = t*16 + lane*8 + b)
P = 128
NEG = -1.0e9

f32 = mybir.dt.float32
bf16 = mybir.dt.bfloat16
AF = mybir.ActivationFunctionType
OP = mybir.AluOpType

ENC_WARM = int(os.environ.get("ENC_WARM", "12"))
PROJ_Q = int(os.environ.get("PROJ_Q", "2"))     # proj jobs per dec step

# vocab projection jobs: pairs of 512-chunks -> 31 x 1024 + 1 x 256
PJOBS = [(i * 1024, 1024) for i in range(31)] + [(31744, 256)]


def _install_profile_hook():
    """Make trace=True work: the image's antenv lacks axon_hooks."""
    if "antenv.axon_hooks" in sys.modules:
        return
    try:
        import trn_agent_boot.trn_boot as tb
        hook = tb._ntff_profile_via_ctypes("/opt/axon/libaxon_pjrt.so")
        m = types.ModuleType("antenv.axon_hooks")
        m.get_axon_ntff_profile_hook = lambda: hook
        m.set_axon_ntff_profile_hook = lambda h: None
        sys.modules["antenv.axon_hooks"] = m
        import antenv
        antenv.axon_hooks = m
        bass_utils.upload_artifacts = lambda d: d
    except Exception:
        pass


# ---------------------------------------------------------------- program
def build_program(dbg=False):
    nc = bacc.Bacc("TRN2", target_bir_lowering=False, debug=False,
                   num_devices=NCORES)

    def din(name, shape, dt=bf16):
        return nc.dram_tensor(name, list(shape), dt, kind="ExternalInput").ap()

    io = {}
    io["xeT_in"] = din("xeT_in", (H, NSCAN))
    io["xdT_in"] = din("xdT_in", (H, NSEQ))
    io["amask"] = din("amask", (Bc, NSEQ), f32)
    for name, shape in [
        ("w0a", (H, H3)), ("u0", (H, H3)), ("bn0", (1, H)),
        ("w1a", (H2, H3)), ("u1", (H, H3)), ("bn1", (1, H)),
        ("fct", (H2, H)), ("fcb", (1, H)),
        ("was", (H2, H)), ("wcc", (H2, H)), ("wch", (H, H)),
        ("wxa", (H, H3)), ("ud", (H, H3)), ("whd", (H, H3)),
        ("bnd", (1, H)),
    ]:
        io[name] = din(name, shape)
    io["b0T"] = din("b0T", (P, 12), f32)
    io["b1T"] = din("b1T", (P, 12), f32)
    io["bdT"] = din("bdT", (P, 12), f32)
    io["owt"] = din("owt", (H, V))
    io["out"] = nc.dram_tensor("out", [NSEQ, V], bf16,
                               kind="ExternalOutput").ap()
    io["dbg"] = dbg
    if dbg:
        io["dbg_xp0T"] = nc.dram_tensor("dbg_xp0T", [P, 12, NSCAN], bf16,
                                        kind="ExternalOutput").ap()
        io["dbg_xpxT"] = nc.dram_tensor("dbg_xpxT", [P, 12, NSEQ], bf16,
                                        kind="ExternalOutput").ap()
        io["dbg_l0T"] = nc.dram_tensor("dbg_l0T", [P, 8, NSCAN], bf16,
                                       kind="ExternalOutput").ap()
        io["dbg_hencT"] = nc.dram_tensor("dbg_hencT", [P, 8, NSEQ], bf16,
                                         kind="ExternalOutput").ap()
        io["dbg_h0"] = nc.dram_tensor("dbg_h0", [Bc, H], f32,
                                      kind="ExternalOutput").ap()
        io["dbg_htall"] = nc.dram_tensor("dbg_htall", [P, 4, NSEQ], bf16,
                                         kind="ExternalOutput").ap()

    with tile.TileContext(nc) as tc:
        _emit(nc, tc, io)
    nc.compile()
    return nc


def _emit(nc, tc, io):
    # ------- pools
    cms = []

    def pool(name, bufs, space=None):
        kw = {"space": space} if space else {}
        cm = tc.tile_pool(name=name, bufs=bufs, **kw)
        cms.append(cm)
        return cm.__enter__()

    cpool = pool("const", 1)
    data = pool("data", 1)
    wgt = pool("wgt", 1)       # weights (tags reused across phases)
    spool = pool("state", 2)
    wpool = pool("work", 2)
    prpool = pool("proj", 3)
    pp = pool("pp", 1, "PSUM")
    ps = pool("ps", 2, "PSUM")
    pt = pool("pt", 2, "PSUM")

    # ---------------- constants
    ident = cpool.tile([P, P], f32)
    make_identity(nc, ident[:])
    identb = cpool.tile([P, P], bf16)
    nc.vector.tensor_copy(identb[:], ident[:])
    onesb = cpool.tile([1, P], bf16)
    nc.vector.memset(onesb[:], 1.0)
    warm_src = cpool.tile([16, H], bf16)
    nc.vector.memset(warm_src[:], 0.125)

    def load_const(name, shape, dt=bf16):
        t = cpool.tile(list(shape), dt, tag=name)
        nc.sync.dma_start(t[:], io[name][:])
        return t

    bn0_sb = load_const("bn0", (1, H))
    bn1_sb = load_const("bn1", (1, H))
    bnd_sb = load_const("bnd", (1, H))
    fcb_sb = load_const("fcb", (1, H))
    b0T_sb = load_const("b0T", (P, 12), f32)
    b1T_sb = load_const("b1T", (P, 12), f32)
    bdT_sb = load_const("bdT", (P, 12), f32)
    amask_sb = cpool.tile([Bc, NSEQ], f32, tag="amask")
    nc.sync.dma_start(amask_sb[:], io["amask"][:])

    # ---------------- persistent SBUF data (xp0/xp1 share slots via tags)
    xp0T = data.tile([P, 12, NSCAN], bf16, tag="xpT")
    xpxT = data.tile([P, 12, NSEQ], bf16)
    l0T = data.tile([P, 8, NSCAN], bf16)     # enc0 output, transposed
    hencT = data.tile([P, 8, NSEQ], bf16)    # enc1 output, transposed
    gT = data.tile([P, 4, NSEQ], bf16)       # Wa . hencT
    pf = data.tile([P, 3, H], bf16)          # henc @ Wcc
    htall = data.tile([P, 4, NSEQ], bf16)    # decoder ht, transposed

    # ---------------- weight loads
    def kload(name, kdim, n, tag, bufs=None):
        ko = kdim // P
        t = wgt.tile([P, ko, n], bf16, tag=tag, bufs=bufs)
        nc.sync.dma_start(t[:], io[name].rearrange("(ko p) n -> p ko n", p=P))
        return t

    xeT = kload("xeT_in", H, NSCAN, "xeT")
    xdT = kload("xdT_in", H, NSEQ, "xdT")
    w0a_sb = kload("w0a", H, H3, "wp")
    wxa_sb = kload("wxa", H, H3, "whd")
    u_sb = kload("u0", H, H3, "u", bufs=2)

    # ---------------- xp pre-GEMMs: transposed layout
    # dst[p, m, r] = (x @ W.T + b)[r, m*128+p]; r/z mtiles 0..7 prescaled .5
    def gemm_T(dst, rhsT, ko, bT, w, nrows):
        i = 0
        for m in range(12):
            for r0 in range(0, nrows, 512):
                rw = min(512, nrows - r0)
                pst = ps.tile([P, 512], f32, tag="sc")
                for k in range(ko):
                    nc.tensor.matmul(pst[:, :rw], w[:, k, m * P:(m + 1) * P],
                                     rhsT[:, k, r0:r0 + rw],
                                     start=(k == 0), stop=(k == ko - 1))
                if i % 2 == 0:
                    nc.scalar.activation(dst[:, m, r0:r0 + rw], pst[:, :rw],
                                         AF.Identity, bias=bT[:, m:m + 1])
                else:
                    nc.vector.tensor_scalar_add(dst[:, m, r0:r0 + rw],
                                                pst[:, :rw], bT[:, m:m + 1])
                i += 1

    gemm_T(xp0T, xeT, 4, b0T_sb, w0a_sb, NSCAN)
    gemm_T(xpxT, xdT, 4, bdT_sb, wxa_sb, NSEQ)
    if io["dbg"]:
        nc.sync.dma_start(io["dbg_xp0T"][:], xp0T[:])
        nc.sync.dma_start(io["dbg_xpxT"][:], xpxT[:])

    # =========================================================== GRU scans
    def warm_mm(m):
        w = ps.tile([P, 512], f32, tag="sc")
        nc.tensor.matmul(w[:m, :], identb[:m, :m], warm_src[:m, :],
                         start=True, stop=True)

    # shared transposed-land tail; returns (h_new_f32, h_new_bf16)
    # pt slices: r=0:4 z=4:8 n=8:12 x=12:16 (x: dec only)
    def gru_tail(m, tp, xpT, s, hT_f, has_x):
        rpre = wpool.tile([P, 4, 16], f32, tag="rpre")
        nc.vector.scalar_tensor_tensor(
            rpre[:, :, :m], tp[:, 0:4, :m], 0.5,
            xpT[:, 0:4, s * m:(s + 1) * m], OP.mult, OP.add)
        tr = wpool.tile([P, 4, 16], f32, tag="tr")
        nc.scalar.activation(tr[:, :, :m], rpre[:, :, :m], AF.Tanh)
        rT = wpool.tile([P, 4, 16], f32, tag="rT")
        nc.vector.tensor_scalar(rT[:, :, :m], tr[:, :, :m], 0.5, 0.5,
                                OP.mult, OP.add)
        rn = wpool.tile([P, 4, 16], f32, tag="rn")
        nc.vector.tensor_mul(rn[:, :, :m], rT[:, :, :m], tp[:, 8:12, :m])
        nin = wpool.tile([P, 4, 16], f32, tag="nin")
        nc.vector.tensor_add(nin[:, :, :m], rn[:, :, :m],
                             xpT[:, 8:12, s * m:(s + 1) * m])
        if has_x:
            nin2 = wpool.tile([P, 4, 16], f32, tag="nin2")
            nc.vector.tensor_add(nin2[:, :, :m], nin[:, :, :m],
                                 tp[:, 12:16, :m])
            nin = nin2
        nT = wpool.tile([P, 4, 16], f32, tag="nT")
        nc.scalar.activation(nT[:, :, :m], nin[:, :, :m], AF.Tanh)
        zpre = wpool.tile([P, 4, 16], f32, tag="zpre")
        nc.vector.scalar_tensor_tensor(
            zpre[:, :, :m], tp[:, 4:8, :m], 0.5,
            xpT[:, 4:8, s * m:(s + 1) * m], OP.mult, OP.add)
        tz = wpool.tile([P, 4, 16], f32, tag="tz")
        nc.scalar.activation(tz[:, :, :m], zpre[:, :, :m], AF.Tanh)
        zT = wpool.tile([P, 4, 16], f32, tag="zT")
        nc.vector.tensor_scalar(zT[:, :, :m], tz[:, :, :m], 0.5, 0.5,
                                OP.mult, OP.add)
        dT = wpool.tile([P, 4, 16], f32, tag="dT")
        nc.vector.tensor_sub(dT[:, :, :m], hT_f[:, :, :m], nT[:, :, :m])
        zd = wpool.tile([P, 4, 16], f32, tag="zd")
        nc.vector.tensor_mul(zd[:, :, :m], zT[:, :, :m], dT[:, :, :m])
        h_new = spool.tile([P, 4, 16], f32, tag="hf")
        nc.vector.tensor_add(h_new[:, :, :m], nT[:, :, :m], zd[:, :, :m])
        hb_new = spool.tile([P, 4, 16], bf16, tag="hb")
        nc.scalar.copy(out=hb_new[:, :, :m], in_=h_new[:, :, :m])
        return h_new, hb_new

    def enc_scan(xpT, u, bn, emit_out):
        hT_f = spool.tile([P, 4, 16], f32, tag="hf")
        hT_b = spool.tile([P, 4, 16], bf16, tag="hb")
        nc.vector.memset(hT_f[:], 0.0)
        nc.vector.memset(hT_b[:], 0.0)
        m = 16
        for s in range(T):
            p = pp.tile([m, 2048], f32, tag="pp")
            # psum regions: r=0:512 z=512:1024 n=1024:1536; emit n, r, z
            nc.tensor.matmul(p[:, 1024:1536], onesb[:1, :m], bn[:1, :],
                             start=True, stop=False)
            for k in range(4):
                nc.tensor.matmul(p[:, 1024:1536], hT_b[:, k, :m],
                                 u[:, k, 1024:1536],
                                 start=False, stop=(k == 3))
            for k in range(4):
                nc.tensor.matmul(p[:, 0:512], hT_b[:, k, :m],
                                 u[:, k, 0:512],
                                 start=(k == 0), stop=(k == 3))
            for k in range(4):
                nc.tensor.matmul(p[:, 512:1024], hT_b[:, k, :m],
                                 u[:, k, 512:1024],
                                 start=(k == 0), stop=(k == 3))
            # crossings (psum -> sbuf, f32)
            cn = wpool.tile([16, H], f32, tag="cn", bufs=1)
            nc.vector.tensor_copy(out=cn[:m, :], in_=p[:, 1024:1536])
            cr = wpool.tile([16, H], f32, tag="cr", bufs=1)
            nc.scalar.copy(out=cr[:m, :], in_=p[:, 0:512])
            cz = wpool.tile([16, H], f32, tag="cz", bufs=1)
            nc.scalar.copy(out=cz[:m, :], in_=p[:, 512:1024])
            # transposes into packed psum tile
            tp = pt.tile([P, 16, 16], f32, tag="tp")
            for k in range(4):
                nc.tensor.transpose(tp[:, 8 + k, :m],
                                    cn[:m, k * P:(k + 1) * P], ident[:m, :m])
            for k in range(4):
                nc.tensor.transpose(tp[:, 0 + k, :m],
                                    cr[:m, k * P:(k + 1) * P], ident[:m, :m])
            for k in range(4):
                nc.tensor.transpose(tp[:, 4 + k, :m],
                                    cz[:m, k * P:(k + 1) * P], ident[:m, :m])
            for _ in range(ENC_WARM):
                warm_mm(m)
            hT_f, hT_b = gru_tail(m, tp, xpT, s, hT_f, False)
            emit_out(s, hT_b)

    def emit_l0(s, hb):
        nc.gpsimd.tensor_copy(out=l0T[:, 0:4, s * 16:s * 16 + 8],
                              in_=hb[:, :, 0:8])
        nc.gpsimd.tensor_copy(
            out=l0T[:, 0:4, (T - 1 - s) * 16 + 8:(T - 1 - s) * 16 + 16],
            in_=hb[:, :, 0:8])
        nc.gpsimd.tensor_copy(
            out=l0T[:, 4:8, (T - 1 - s) * 16:(T - 1 - s) * 16 + 8],
            in_=hb[:, :, 8:16])
        nc.gpsimd.tensor_copy(out=l0T[:, 4:8, s * 16 + 8:s * 16 + 16],
                              in_=hb[:, :, 8:16])

    def emit_henc(s, hb):
        nc.gpsimd.tensor_copy(out=hencT[:, 0:4, s * 8:s * 8 + 8],
                              in_=hb[:, :, 0:8])
        nc.gpsimd.tensor_copy(
            out=hencT[:, 4:8, (T - 1 - s) * 8:(T - 1 - s) * 8 + 8],
            in_=hb[:, :, 8:16])

    enc_scan(xp0T, u_sb, bn0_sb, emit_l0)

    # xp1 GEMM (rhsT = l0T), then enc1; xp1 reuses the xp0 slot
    w1a_sb = kload("w1a", H2, H3, "wp")
    xp1T = data.tile([P, 12, NSCAN], bf16, tag="xpT")
    gemm_T(xp1T, l0T, 8, b1T_sb, w1a_sb, NSCAN)
    u1_sb = kload("u1", H, H3, "u", bufs=2)
    enc_scan(xp1T, u1_sb, bn1_sb, emit_henc)

    # =========================================================== attn pre
    was_sb = kload("was", H2, H, "wp")
    for mth in range(4):
        pst = ps.tile([P, 512], f32, tag="sc")
        for k in range(8):
            nc.tensor.matmul(pst[:, :NSEQ], was_sb[:, k, mth * P:(mth + 1) * P],
                             hencT[:, k, :], start=(k == 0), stop=(k == 7))
        nc.vector.tensor_copy(out=gT[:, mth, :], in_=pst[:, :NSEQ])
    wcc_sb = kload("wcc", H2, H, "wpn")
    for mth in range(3):
        pst = ps.tile([P, 512], f32, tag="sc")
        for k in range(8):
            nc.tensor.matmul(pst[:, :H], hencT[:, k, mth * P:(mth + 1) * P],
                             wcc_sb[:, k, :], start=(k == 0), stop=(k == 7))
        nc.vector.tensor_copy(out=pf[:, mth, :], in_=pst[:, :H])
    fct_sb = kload("fct", H2, H, "wp")
    h0p = ps.tile([P, 512], f32, tag="sc")
    for k in range(8):
        c0 = (T - 1) * Bc if k < 4 else 0
        nc.tensor.matmul(h0p[:Bc, :H], hencT[:, k, c0:c0 + Bc],
                         fct_sb[:, k, :], start=(k == 0), stop=False)
    nc.tensor.matmul(h0p[:Bc, :H], onesb[:1, :Bc], fcb_sb[:1, :],
                     start=False, stop=True)
    h0v = wpool.tile([Bc, H], f32, tag="h0v", bufs=1)
    nc.scalar.activation(h0v[:], h0p[:Bc, :H], AF.Tanh)
    if io["dbg"]:
        nc.sync.dma_start(io["dbg_h0"][:], h0v[:])
    tp0 = pt.tile([P, 16, 16], f32, tag="tp")
    for k in range(4):
        nc.tensor.transpose(tp0[:, k, :Bc], h0v[:, k * P:(k + 1) * P],
                            ident[:Bc, :Bc])
    h0T_f = spool.tile([P, 4, 16], f32, tag="hf")
    nc.vector.tensor_copy(out=h0T_f[:, :, :Bc], in_=tp0[:, 0:4, :Bc])
    h0T_b = spool.tile([P, 4, 16], bf16, tag="hb")
    nc.scalar.copy(out=h0T_b[:, :, :Bc], in_=tp0[:, 0:4, :Bc])

    # =========================================================== decoder
    ud_sb = kload("ud", H, H3, "u", bufs=2)
    whd_sb = kload("whd", H, H3, "whd")
    wch_sb = kload("wch", H, H, "wpn")
    owt_r = io["owt"].rearrange("(ko p) v -> p ko v", p=P)

    proj_jobs = [(0, c0, cw) for c0, cw in PJOBS] + \
                [(1, c0, cw) for c0, cw in PJOBS] + \
                [(2, c0, cw) for c0, cw in PJOBS]
    pj_state = {"i": 0, "eng": 0}

    def emit_proj(budget, avail_m):
        while budget > 0 and pj_state["i"] < len(proj_jobs):
            mth, c0, cw = proj_jobs[pj_state["i"]]
            if mth > avail_m:
                return
            pj_state["i"] += 1
            budget -= 1
            ow = prpool.tile([P, 4, 1024], bf16, tag="ow", bufs=2)
            nc.sync.dma_start(ow[:, :, :cw], owt_r[:, :, c0:c0 + cw])
            ob = prpool.tile([P, 1024], bf16, tag="ob")
            for v0 in range(0, cw, 512):
                vw = min(512, cw - v0)
                pst = ps.tile([P, 512], f32, tag="sc")
                for k in range(4):
                    nc.tensor.matmul(pst[:, :vw],
                                     htall[:, k, mth * P:(mth + 1) * P],
                                     ow[:, k, v0:v0 + vw],
                                     start=(k == 0), stop=(k == 3))
                if pj_state["eng"] % 2 == 0:
                    nc.scalar.copy(out=ob[:, v0:v0 + vw], in_=pst[:, :vw])
                else:
                    nc.vector.tensor_copy(out=ob[:, v0:v0 + vw],
                                          in_=pst[:, :vw])
                pj_state["eng"] += 1
            nc.sync.dma_start(io["out"][mth * P:(mth + 1) * P, c0:c0 + cw],
                              ob[:, :cw])

    hT_f, hT_b = h0T_f, h0T_b
    m = Bc
    for t in range(T):
        p = pp.tile([m, 2048], f32, tag="pp")
        # regions: r=0:512 z=512:1024 nh=1024:1536 nx=1536:2048
        # u-part first (depends only on h)
        nc.tensor.matmul(p[:, 1024:1536], onesb[:1, :m], bnd_sb[:1, :],
                         start=True, stop=False)
        for k in range(4):
            nc.tensor.matmul(p[:, 1024:1536], hT_b[:, k, :m],
                             ud_sb[:, k, 1024:1536],
                             start=False, stop=(k == 3))
        for k in range(4):
            nc.tensor.matmul(p[:, 0:512], hT_b[:, k, :m], ud_sb[:, k, 0:512],
                             start=(k == 0), stop=(t == 0 and k == 3))
        for k in range(4):
            nc.tensor.matmul(p[:, 512:1024], hT_b[:, k, :m],
                             ud_sb[:, k, 512:1024],
                             start=(k == 0), stop=(t == 0 and k == 3))
        # wh-part (depends on ht from previous step)
        if t > 0:
            htprev = htall[:, :, (t - 1) * m:t * m]
            for k in range(4):
                nc.tensor.matmul(p[:, 1536:2048], htprev[:, k, :],
                                 whd_sb[:, k, 1024:1536],
                                 start=(k == 0), stop=(k == 3))
            for k in range(4):
                nc.tensor.matmul(p[:, 0:512], htprev[:, k, :],
                                 whd_sb[:, k, 0:512],
                                 start=False, stop=(k == 3))
            for k in range(4):
                nc.tensor.matmul(p[:, 512:1024], htprev[:, k, :],
                                 whd_sb[:, k, 512:1024],
                                 start=False, stop=(k == 3))
        # crossings
        cn = wpool.tile([16, H], f32, tag="cn", bufs=1)
        nc.vector.tensor_copy(out=cn[:m, :], in_=p[:, 1024:1536])
        if t > 0:
            cx = wpool.tile([16, H], f32, tag="cx", bufs=1)
            nc.vector.tensor_copy(out=cx[:m, :], in_=p[:, 1536:2048])
        cr = wpool.tile([16, H], f32, tag="cr", bufs=1)
        nc.scalar.copy(out=cr[:m, :], in_=p[:, 0:512])
        cz = wpool.tile([16, H], f32, tag="cz", bufs=1)
        nc.scalar.copy(out=cz[:m, :], in_=p[:, 512:1024])
        tp = pt.tile([P, 16, 16], f32, tag="tp")
        for k in range(4):
            nc.tensor.transpose(tp[:, 8 + k, :m], cn[:m, k * P:(k + 1) * P],
                                ident[:m, :m])
        if t > 0:
            for k in range(4):
                nc.tensor.transpose(tp[:, 12 + k, :m],
                                    cx[:m, k * P:(k + 1) * P], ident[:m, :m])
        for k in range(4):
            nc.tensor.transpose(tp[:, 0 + k, :m], cr[:m, k * P:(k + 1) * P],
                                ident[:m, :m])
        for k in range(4):
            nc.tensor.transpose(tp[:, 4 + k, :m], cz[:m, k * P:(k + 1) * P],
                                ident[:m, :m])
        hT_f, hT_b = gru_tail(m, tp, xpxT, t, hT_f, t > 0)
        hb_new = hT_b
        # ---- attention
        pst_sc = ps.tile([P, 512], f32, tag="sc")
        for k in range(4):
            nc.tensor.matmul(pst_sc[:m, :NSEQ], hb_new[:, k, :m], gT[:, k, :],
                             start=(k == 0), stop=(k == 3))
        sc2 = wpool.tile([Bc, NSEQ], f32, tag="sc2")
        nc.vector.tensor_add(sc2[:], pst_sc[:m, :NSEQ], amask_sb[:])
        alpha = wpool.tile([Bc, NSEQ], f32, tag="alpha")
        sexp = wpool.tile([Bc, 1], f32, tag="sexp")
        nc.scalar.activation(alpha[:], sc2[:], AF.Exp, accum_out=sexp[:])
        rs = wpool.tile([Bc, 1], f32, tag="rs")
        nc.vector.reciprocal(rs[:], sexp[:])
        alpha2 = wpool.tile([Bc, NSEQ], f32, tag="alpha2")
        nc.vector.tensor_scalar_mul(alpha2[:], alpha[:], rs[:])
        tpa = pt.tile([P, 16, 16], f32, tag="tp")
        for j in range(3):
            nc.tensor.transpose(tpa[:, j, :m], alpha2[:, j * P:(j + 1) * P],
                                ident[:m, :m])
        aT = wpool.tile([P, 3, Bc], bf16, tag="aT")
        nc.vector.tensor_copy(out=aT[:], in_=tpa[:, 0:3, :m])
        emit_proj(1 if t >= 16 else 0, (t - 16) // 16)
        htp = ps.tile([P, 512], f32, tag="sc")
        for j in range(3):
            nc.tensor.matmul(htp[:m, :H], aT[:, j, :], pf[:, j, :],
                             start=(j == 0), stop=False)
        for k in range(4):
            nc.tensor.matmul(htp[:m, :H], hb_new[:, k, :m], wch_sb[:, k, :],
                             start=False, stop=(k == 3))
        htv = wpool.tile([Bc, H], f32, tag="htv")
        nc.scalar.activation(htv[:], htp[:m, :H], AF.Tanh)
        tph = pt.tile([P, 16, 16], f32, tag="tp")
        for k in range(4):
            nc.tensor.transpose(tph[:, k, :m], htv[:, k * P:(k + 1) * P],
                                ident[:m, :m])
        nc.scalar.copy(out=htall[:, :, t * m:(t + 1) * m],
                       in_=tph[:, 0:4, :m])
        emit_proj(PROJ_Q - 1 if t >= 16 else 0, (t - 16) // 16)

    # drain remaining projection jobs
    emit_proj(len(proj_jobs), 2)

    if io["dbg"]:
        nc.sync.dma_start(io["dbg_l0T"][:], l0T[:])
        nc.sync.dma_start(io["dbg_hencT"][:], hencT[:])
        nc.sync.dma_start(io["dbg_htall"][:], htall[:])

    for cm in reversed(cms):
        cm.__exit__(None, None, None)


# ---------------------------------------------------------------- host side
_PROGRAM = None


def _get_program():
    global _PROGRAM
    if _PROGRAM is None:
        _install_profile_hook()
        _PROGRAM = build_program(dbg=os.environ.get("KDBG", "0") == "1")
    return _PROGRAM


def _prep_shared(inputs):
    import ml_dtypes
    f = np.float32
    bf = ml_dtypes.bfloat16

    def b(x):
        return np.ascontiguousarray(np.asarray(x, f).astype(bf))

    def wall(wih, bih):
        # full Wih.T with r/z columns prescaled by 0.5; bias transposed
        wt = np.asarray(wih, f).T.copy()           # (in, 3H)
        bb = np.asarray(bih, f).copy()             # (3H,)
        wt[:, 0:H2] *= 0.5
        bb2 = bb.copy()
        bb2[0:H2] *= 0.5
        bT = np.ascontiguousarray(bb2.reshape(12, P).T)   # (128, 12) f32
        return b(wt), bT

    g = {}
    g["w0a"], g["b0T"] = wall(inputs["enc0_Wih"], inputs["enc0_bih"])
    g["u0"] = b(np.concatenate([np.asarray(inputs["enc0_Ur"], f).T,
                                np.asarray(inputs["enc0_Uz"], f).T,
                                np.asarray(inputs["enc0_Un"], f).T], axis=1))
    g["bn0"] = b(np.asarray(inputs["enc0_bn"], f)[None, :])
    g["w1a"], g["b1T"] = wall(inputs["enc1_Wih"], inputs["enc1_bih"])
    g["u1"] = b(np.concatenate([np.asarray(inputs["enc1_Ur"], f).T,
                                np.asarray(inputs["enc1_Uz"], f).T,
                                np.asarray(inputs["enc1_Un"], f).T], axis=1))
    g["bn1"] = b(np.asarray(inputs["enc1_bn"], f)[None, :])
    g["fct"] = b(np.asarray(inputs["fc_init_w"], f).T)
    g["fcb"] = b(np.asarray(inputs["fc_init_b"], f)[None, :])
    scale = np.float32(1.0) / np.sqrt(np.float32(H2))
    g["was"] = b(np.asarray(inputs["Wa"], f) * scale)
    acw = np.asarray(inputs["attn_combine_w"], f)
    g["wch"] = b(acw[:, :H].T)
    g["wcc"] = b(acw[:, H:].T)
    dwih = np.asarray(inputs["dec_Wih"], f)
    g["wxa"], g["bdT"] = wall(dwih[:, :H], inputs["dec_bih"])
    g["whd"] = b(dwih[:, H:].T)
    g["ud"] = b(np.concatenate([np.asarray(inputs["dec_Ur"], f).T,
                                np.asarray(inputs["dec_Uz"], f).T,
                                np.asarray(inputs["dec_Un"], f).T], axis=1))
    g["bnd"] = b(np.asarray(inputs["dec_bn"], f)[None, :])
    g["owt"] = b(np.asarray(inputs["out_w"], f).T)
    return g


def _prep_core(inputs, c):
    import ml_dtypes
    bf = ml_dtypes.bfloat16
    src = np.asarray(inputs["src"])
    tgt = np.asarray(inputs["tgt"])
    emb = np.asarray(inputs["emb"], np.float32)
    si = src[:, c * Bc:(c + 1) * Bc].astype(np.int64)      # (48, 8)
    ti = tgt[:, c * Bc:(c + 1) * Bc].astype(np.int64)
    idx_enc = np.empty((T, 2, Bc), np.int64)
    idx_enc[:, 0, :] = si
    idx_enc[:, 1, :] = si[::-1]
    xeT_in = np.ascontiguousarray(emb[idx_enc.reshape(NSCAN)].T.astype(bf))
    xdT_in = np.ascontiguousarray(emb[ti.reshape(NSEQ)].T.astype(bf))
    mk = np.full((Bc, T, Bc), NEG, np.float32)
    for bb in range(Bc):
        mk[bb, :, bb] = np.where(si[:, bb] != 0, np.float32(0.0),
                                 np.float32(NEG))
    return {"xeT_in": xeT_in,
            "xdT_in": xdT_in,
            "amask": mk.reshape(Bc, NSEQ)}


def kernel(**inputs):
    nc = _get_program()
    shared = _prep_shared(inputs)
    in_maps = []
    for c in range(NCORES):
        im = dict(shared)
        im.update(_prep_core(inputs, c))
        in_maps.append(im)
    res = run_bass_kernel_spmd(nc, in_maps, core_ids=list(range(NCORES)))
    logits = np.empty((T, B, V), np.float32)
    for c in range(NCORES):
        logits[:, c * Bc:(c + 1) * Bc, :] = \
            res.results[c]["out"].astype(np.float32).reshape(T, Bc, V)
    return logits


# revision 18
# speedup vs baseline: 1.3660x; 1.2141x over previous
"""Luong seq2seq (2-layer BiGRU encoder + attention GRU decoder + vocab
projection) as a single 8-core SPMD Bass/Tile kernel for Trainium2.

v2: bf16 matmuls everywhere, transposed-land elementwise tail (ops on
[128, 4*m] instead of [m, 512]), sigmoid-via-tanh (single activation
table, r/z input projections pre-scaled by 0.5 on the host), SBUF-
resident scratch with incremental l0T/hencT build (no per-step DMAs,
no transpose phases), greedy interleaved vocab projection, bf16 logits
output (host casts to f32).

Sharding: data-parallel over batch (64 -> 8 per core); each core
projects onto the full 32000-word vocab.

Self-contained: hardcodes all shapes; takes the full unsharded inputs
of reference.setup_inputs() and returns the full (48, 64, 32000) f32
logits.
"""

import os
import sys
import types

for _p in ("/opt/trn_rl_repo", "/opt/pypackages", "/root/.axon_site",
           "/root/.axon_site/_ro/trn_rl_repo", "/root/.axon_site/_ro/pypackages"):
    if os.path.isdir(_p) and _p not in sys.path:
        sys.path.append(_p)

import numpy as np

from concourse import bass, mybir, tile, bacc
from concourse import bass_utils
from concourse.bass_utils import run_bass_kernel_spmd
from concourse.masks import make_identity

# ---------------------------------------------------------------- constants
V, H, T, B, NCORES = 32000, 512, 48, 64, 8
Bc = B // NCORES            # 8 examples per core
H2, H3 = 2 * H, 3 * H
NSEQ = T * Bc               # 384 decoder rows (t-major: r = t*Bc + b)
NSCAN = T * 2 * Bc          # 768 encoder scan rows (r = t*16 + lane*8 + b)
P = 128
NEG = -1.0e9

f32 = mybir.dt.float32
bf16 = mybir.dt.bfloat16
AF = mybir.ActivationFunctionType
OP = mybir.AluOpType

ENC_WARM = int(os.environ.get("ENC_WARM", "0"))
PROJ_Q = int(os.environ.get("PROJ_Q", "2"))     # proj jobs per dec step

# vocab projection jobs: pairs of 512-chunks -> 31 x 1024 + 1 x 256
PJOBS = [(i * 1024, 1024) for i in range(31)] + [(31744, 256)]


def _install_profile_hook():
    """Make trace=True work: the image's antenv lacks axon_hooks."""
    if "antenv.axon_hooks" in sys.modules:
        return
    try:
        import trn_agent_boot.trn_boot as tb
        hook = tb._ntff_profile_via_ctypes("/opt/axon/libaxon_pjrt.so")
        m = types.ModuleType("antenv.axon_hooks")
        m.get_axon_ntff_profile_hook = lambda: hook
        m.set_axon_ntff_profile_hook = lambda h: None
        sys.modules["antenv.axon_hooks"] = m
        import antenv
        antenv.axon_hooks = m
        bass_utils.upload_artifacts = lambda d: d
    except Exception:
        pass


# ---------------------------------------------------------------- program
def build_program(dbg=False):
    nc = bacc.Bacc("TRN2", target_bir_lowering=False, debug=False,
                   num_devices=NCORES)

    def din(name, shape, dt=bf16):
        return nc.dram_tensor(name, list(shape), dt, kind="ExternalInput").ap()

    io = {}
    io["xeT_in"] = din("xeT_in", (H, NSCAN))
    io["xdT_in"] = din("xdT_in", (H, NSEQ))
    io["amask"] = din("amask", (Bc, NSEQ))
    for name, shape in [
        ("w0a", (H, H3)), ("u0", (H, H3)), ("bn0", (1, H)),
        ("w1a", (H2, H3)), ("u1", (H, H3)), ("bn1", (1, H)),
        ("fct", (H2, H)), ("fcb", (1, H)),
        ("was", (H2, H)), ("wcc", (H2, H)), ("wch", (H, H)),
        ("wxa", (H, H3)), ("ud", (H, H3)), ("whd", (H, H3)),
        ("bnd", (1, H)),
    ]:
        io[name] = din(name, shape)
    io["b0T"] = din("b0T", (P, 12), f32)
    io["b1T"] = din("b1T", (P, 12), f32)
    io["bdT"] = din("bdT", (P, 12), f32)
    io["owt"] = din("owt", (H, V))
    io["out"] = nc.dram_tensor("out", [NSEQ, V], bf16,
                               kind="ExternalOutput").ap()
    io["dbg"] = dbg
    if dbg:
        io["dbg_xp0T"] = nc.dram_tensor("dbg_xp0T", [P, 12, NSCAN], bf16,
                                        kind="ExternalOutput").ap()
        io["dbg_xpxT"] = nc.dram_tensor("dbg_xpxT", [P, 12, NSEQ], bf16,
                                        kind="ExternalOutput").ap()
        io["dbg_l0T"] = nc.dram_tensor("dbg_l0T", [P, 8, NSCAN], bf16,
                                       kind="ExternalOutput").ap()
        io["dbg_hencT"] = nc.dram_tensor("dbg_hencT", [P, 8, NSEQ], bf16,
                                         kind="ExternalOutput").ap()
        io["dbg_h0"] = nc.dram_tensor("dbg_h0", [Bc, H], f32,
                                      kind="ExternalOutput").ap()
        io["dbg_htall"] = nc.dram_tensor("dbg_htall", [P, 4, NSEQ], bf16,
                                         kind="ExternalOutput").ap()

    with tile.TileContext(nc) as tc:
        _emit(nc, tc, io)
    nc.compile()
    return nc


def _emit(nc, tc, io):
    # ------- pools
    cms = []

    def pool(name, bufs, space=None):
        kw = {"space": space} if space else {}
        cm = tc.tile_pool(name=name, bufs=bufs, **kw)
        cms.append(cm)
        return cm.__enter__()

    cpool = pool("const", 1)
    data = pool("data", 1)
    wgt = pool("wgt", 1)       # weights (tags reused across phases)
    spool = pool("state", 2)
    wpool = pool("work", 2)
    prpool = pool("proj", 3)
    pq = pool("pq", 1, "PSUM")   # per-region gate psums (4 banks)
    ps = pool("ps", 2, "PSUM")
    pt = pool("pt", 2, "PSUM")

    # ---------------- constants
    ident = cpool.tile([P, P], f32)
    make_identity(nc, ident[:])
    identb = cpool.tile([P, P], bf16)
    nc.vector.tensor_copy(identb[:], ident[:])
    onesb = cpool.tile([1, P], bf16)
    nc.vector.memset(onesb[:], 1.0)

    def load_const(name, shape, dt=bf16):
        t = cpool.tile(list(shape), dt, tag=name)
        nc.sync.dma_start(t[:], io[name][:])
        return t

    bn0_sb = load_const("bn0", (1, H))
    bn1_sb = load_const("bn1", (1, H))
    bnd_sb = load_const("bnd", (1, H))
    fcb_sb = load_const("fcb", (1, H))
    b0T_sb = load_const("b0T", (P, 12), f32)
    b1T_sb = load_const("b1T", (P, 12), f32)
    bdT_sb = load_const("bdT", (P, 12), f32)
    amask_sb = load_const("amask", (Bc, NSEQ))

    # ---------------- persistent SBUF data (xp0/xp1 share slots via tags)
    xp0T = data.tile([P, 12, NSCAN], bf16, tag="xpT")
    xpxT = data.tile([P, 12, NSEQ], bf16)
    l0T = data.tile([P, 8, NSCAN], bf16)     # enc0 output, transposed
    hencT = data.tile([P, 8, NSEQ], bf16)    # enc1 output, transposed
    gT = data.tile([P, 4, NSEQ], bf16)       # Wa . hencT
    pf = data.tile([P, 3, H], bf16)          # henc @ Wcc
    htall = data.tile([P, 4, NSEQ], bf16)    # decoder ht, transposed

    # ---------------- weight loads
    def kload(name, kdim, n, tag, bufs=None):
        ko = kdim // P
        t = wgt.tile([P, ko, n], bf16, tag=tag, bufs=bufs)
        nc.sync.dma_start(t[:], io[name].rearrange("(ko p) n -> p ko n", p=P))
        return t

    xeT = kload("xeT_in", H, NSCAN, "xeT")
    xdT = kload("xdT_in", H, NSEQ, "xdT")
    w0a_sb = kload("w0a", H, H3, "wp")
    wxa_sb = kload("wxa", H, H3, "whd")
    u_sb = kload("u0", H, H3, "u", bufs=2)

    # ---------------- xp pre-GEMM jobs (transposed layout)
    # dst[p, m, r] = (x @ W.T + b)[r, m*128+p]; r/z mtiles 0..7 prescaled .5
    gjob_i = [0]

    def gemm_T_job(dst, rhsT, ko, bT, w, m, r0, rw):
        pst = ps.tile([P, 512], f32, tag="sc")
        for k in range(ko):
            nc.tensor.matmul(pst[:, :rw], w[:, k, m * P:(m + 1) * P],
                             rhsT[:, k, r0:r0 + rw],
                             start=(k == 0), stop=(k == ko - 1))
        if gjob_i[0] % 2 == 0:
            nc.scalar.activation(dst[:, m, r0:r0 + rw], pst[:, :rw],
                                 AF.Identity, bias=bT[:, m:m + 1])
        else:
            nc.vector.tensor_scalar_add(dst[:, m, r0:r0 + rw],
                                        pst[:, :rw], bT[:, m:m + 1])
        gjob_i[0] += 1

    # chunks needed before a scan starts (rows 0:512 = steps 0..31)
    for m in range(12):
        gemm_T_job(xp0T, xeT, 4, b0T_sb, w0a_sb, m, 0, 512)
    # fillers: late xp0 chunks + the whole decoder xp
    enc0_fill = [(xp0T, xeT, 4, b0T_sb, w0a_sb, m, 512, 256)
                 for m in range(12)]
    enc0_fill += [(xpxT, xdT, 4, bdT_sb, wxa_sb, m, 0, NSEQ)
                  for m in range(12)]

    # =========================================================== GRU scans
    # shared transposed-land tail; pt slices: r=0:4 z=4:8 n=8:12 x=12:16
    def gru_tail(m, tp, xpT, s, hT_f, has_x):
        rzp = wpool.tile([P, 8, 16], f32, tag="rzp")
        nc.vector.scalar_tensor_tensor(
            rzp[:, :, :m], tp[:, 0:8, :m], 0.5,
            xpT[:, 0:8, s * m:(s + 1) * m], OP.mult, OP.add)
        trz = wpool.tile([P, 8, 16], f32, tag="trz")
        nc.scalar.activation(trz[:, :, :m], rzp[:, :, :m], AF.Tanh)
        rz = wpool.tile([P, 8, 16], f32, tag="rz")
        nc.vector.tensor_scalar(rz[:, :, :m], trz[:, :, :m], 0.5, 0.5,
                                OP.mult, OP.add)
        rn = wpool.tile([P, 4, 16], f32, tag="rn")
        nc.vector.tensor_mul(rn[:, :, :m], rz[:, 0:4, :m], tp[:, 8:12, :m])
        nin = wpool.tile([P, 4, 16], f32, tag="nin")
        nc.vector.tensor_add(nin[:, :, :m], rn[:, :, :m],
                             xpT[:, 8:12, s * m:(s + 1) * m])
        if has_x:
            nin2 = wpool.tile([P, 4, 16], f32, tag="nin2")
            nc.vector.tensor_add(nin2[:, :, :m], nin[:, :, :m],
                                 tp[:, 12:16, :m])
            nin = nin2
        nT = wpool.tile([P, 4, 16], f32, tag="nT")
        nc.scalar.activation(nT[:, :, :m], nin[:, :, :m], AF.Tanh)
        dT = wpool.tile([P, 4, 16], f32, tag="dT")
        nc.vector.tensor_sub(dT[:, :, :m], hT_f[:, :, :m], nT[:, :, :m])
        zd = wpool.tile([P, 4, 16], f32, tag="zd")
        nc.vector.tensor_mul(zd[:, :, :m], rz[:, 4:8, :m], dT[:, :, :m])
        h_new = spool.tile([P, 4, 16], f32, tag="hf")
        nc.vector.tensor_add(h_new[:, :, :m], nT[:, :, :m], zd[:, :, :m])
        hb_new = spool.tile([P, 4, 16], bf16, tag="hb")
        nc.scalar.copy(out=hb_new[:, :, :m], in_=h_new[:, :, :m])
        return h_new, hb_new

    def enc_scan(xpT, u, bn, emit_out, fillers):
        hT_f = spool.tile([P, 4, 16], f32, tag="hf")
        hT_b = spool.tile([P, 4, 16], bf16, tag="hb")
        nc.vector.memset(hT_f[:], 0.0)
        nc.vector.memset(hT_b[:], 0.0)
        m = 16
        for s in range(T):
            ppr = pq.tile([16, 512], f32, tag="ppr")
            ppz = pq.tile([16, 512], f32, tag="ppz")
            ppn = pq.tile([16, 512], f32, tag="ppn")
            for k in range(4):
                nc.tensor.matmul(ppr[:m, :], hT_b[:, k, :m], u[:, k, 0:512],
                                 start=(k == 0), stop=(k == 3))
            for k in range(4):
                nc.tensor.matmul(ppz[:m, :], hT_b[:, k, :m],
                                 u[:, k, 512:1024],
                                 start=(k == 0), stop=(k == 3))
            nc.tensor.matmul(ppn[:m, :], onesb[:1, :m], bn[:1, :],
                             start=True, stop=False)
            for k in range(4):
                nc.tensor.matmul(ppn[:m, :], hT_b[:, k, :m],
                                 u[:, k, 1024:1536],
                                 start=False, stop=(k == 3))
            # crossings (psum -> sbuf, f32)
            cr = wpool.tile([16, H], f32, tag="cr", bufs=1)
            nc.scalar.copy(out=cr[:m, :], in_=ppr[:m, :])
            cz = wpool.tile([16, H], f32, tag="cz", bufs=1)
            nc.scalar.copy(out=cz[:m, :], in_=ppz[:m, :])
            cn = wpool.tile([16, H], f32, tag="cn", bufs=1)
            nc.vector.tensor_copy(out=cn[:m, :], in_=ppn[:m, :])
            # PE filler while crossings run
            if fillers:
                gemm_T_job(*fillers.pop(0))
            # transposes into packed psum tile
            tp = pt.tile([P, 16, 16], f32, tag="tp")
            for k in range(4):
                nc.tensor.transpose(tp[:, 0 + k, :m],
                                    cr[:m, k * P:(k + 1) * P], ident[:m, :m])
            for k in range(4):
                nc.tensor.transpose(tp[:, 4 + k, :m],
                                    cz[:m, k * P:(k + 1) * P], ident[:m, :m])
            for k in range(4):
                nc.tensor.transpose(tp[:, 8 + k, :m],
                                    cn[:m, k * P:(k + 1) * P], ident[:m, :m])
            if fillers:
                gemm_T_job(*fillers.pop(0))
            hT_f, hT_b = gru_tail(m, tp, xpT, s, hT_f, False)
            emit_out(s, hT_b)

    def emit_l0(s, hb):
        nc.gpsimd.tensor_copy(out=l0T[:, 0:4, s * 16:s * 16 + 8],
                              in_=hb[:, :, 0:8])
        nc.gpsimd.tensor_copy(
            out=l0T[:, 0:4, (T - 1 - s) * 16 + 8:(T - 1 - s) * 16 + 16],
            in_=hb[:, :, 0:8])
        nc.gpsimd.tensor_copy(
            out=l0T[:, 4:8, (T - 1 - s) * 16:(T - 1 - s) * 16 + 8],
            in_=hb[:, :, 8:16])
        nc.gpsimd.tensor_copy(out=l0T[:, 4:8, s * 16 + 8:s * 16 + 16],
                              in_=hb[:, :, 8:16])

    def emit_henc(s, hb):
        nc.gpsimd.tensor_copy(out=hencT[:, 0:4, s * 8:s * 8 + 8],
                              in_=hb[:, :, 0:8])
        nc.gpsimd.tensor_copy(
            out=hencT[:, 4:8, (T - 1 - s) * 8:(T - 1 - s) * 8 + 8],
            in_=hb[:, :, 8:16])

    enc_scan(xp0T, u_sb, bn0_sb, emit_l0, enc0_fill)
    if io["dbg"]:
        nc.sync.dma_start(io["dbg_xp0T"][:], xp0T[:])
        nc.sync.dma_start(io["dbg_xpxT"][:], xpxT[:])

    # xp1 GEMM (rhsT = l0T), then enc1; xp1 reuses the xp0 slot
    w1a_sb = kload("w1a", H2, H3, "wp")
    xp1T = data.tile([P, 12, NSCAN], bf16, tag="xpT")
    for m in range(12):
        gemm_T_job(xp1T, l0T, 8, b1T_sb, w1a_sb, m, 0, 512)
    enc1_fill = [(xp1T, l0T, 8, b1T_sb, w1a_sb, m, 512, 256)
                 for m in range(12)]
    u1_sb = kload("u1", H, H3, "u", bufs=2)
    enc_scan(xp1T, u1_sb, bn1_sb, emit_henc, enc1_fill)

    # =========================================================== attn pre
    was_sb = kload("was", H2, H, "wp")
    for mth in range(4):
        pst = ps.tile([P, 512], f32, tag="sc")
        for k in range(8):
            nc.tensor.matmul(pst[:, :NSEQ], was_sb[:, k, mth * P:(mth + 1) * P],
                             hencT[:, k, :], start=(k == 0), stop=(k == 7))
        nc.vector.tensor_copy(out=gT[:, mth, :], in_=pst[:, :NSEQ])
    wcc_sb = kload("wcc", H2, H, "wpn")
    for mth in range(3):
        pst = ps.tile([P, 512], f32, tag="sc")
        for k in range(8):
            nc.tensor.matmul(pst[:, :H], hencT[:, k, mth * P:(mth + 1) * P],
                             wcc_sb[:, k, :], start=(k == 0), stop=(k == 7))
        nc.vector.tensor_copy(out=pf[:, mth, :], in_=pst[:, :H])
    fct_sb = kload("fct", H2, H, "wp")
    h0p = ps.tile([P, 512], f32, tag="sc")
    for k in range(8):
        c0 = (T - 1) * Bc if k < 4 else 0
        nc.tensor.matmul(h0p[:Bc, :H], hencT[:, k, c0:c0 + Bc],
                         fct_sb[:, k, :], start=(k == 0), stop=False)
    nc.tensor.matmul(h0p[:Bc, :H], onesb[:1, :Bc], fcb_sb[:1, :],
                     start=False, stop=True)
    h0v = wpool.tile([Bc, H], f32, tag="h0v", bufs=1)
    nc.scalar.activation(h0v[:], h0p[:Bc, :H], AF.Tanh)
    if io["dbg"]:
        nc.sync.dma_start(io["dbg_h0"][:], h0v[:])
    tp0 = pt.tile([P, 16, 16], f32, tag="tp")
    for k in range(4):
        nc.tensor.transpose(tp0[:, k, :Bc], h0v[:, k * P:(k + 1) * P],
                            ident[:Bc, :Bc])
    h0T_f = spool.tile([P, 4, 16], f32, tag="hf")
    nc.vector.tensor_copy(out=h0T_f[:, :, :Bc], in_=tp0[:, 0:4, :Bc])
    h0T_b = spool.tile([P, 4, 16], bf16, tag="hb")
    nc.scalar.copy(out=h0T_b[:, :, :Bc], in_=tp0[:, 0:4, :Bc])

    # =========================================================== decoder
    ud_sb = kload("ud", H, H3, "u", bufs=2)
    whd_sb = kload("whd", H, H3, "whd")
    wch_sb = kload("wch", H, H, "wpn")
    owt_r = io["owt"].rearrange("(ko p) v -> p ko v", p=P)

    proj_jobs = [(0, c0, cw) for c0, cw in PJOBS] + \
                [(1, c0, cw) for c0, cw in PJOBS] + \
                [(2, c0, cw) for c0, cw in PJOBS]
    pj_state = {"i": 0, "eng": 0}

    def emit_proj(budget, avail_m):
        while budget > 0 and pj_state["i"] < len(proj_jobs):
            mth, c0, cw = proj_jobs[pj_state["i"]]
            if mth > avail_m:
                return
            pj_state["i"] += 1
            budget -= 1
            ow = prpool.tile([P, 4, 1024], bf16, tag="ow", bufs=2)
            nc.sync.dma_start(ow[:, :, :cw], owt_r[:, :, c0:c0 + cw])
            ob = prpool.tile([P, 1024], bf16, tag="ob")
            for v0 in range(0, cw, 512):
                vw = min(512, cw - v0)
                pst = ps.tile([P, 512], f32, tag="sc")
                for k in range(4):
                    nc.tensor.matmul(pst[:, :vw],
                                     htall[:, k, mth * P:(mth + 1) * P],
                                     ow[:, k, v0:v0 + vw],
                                     start=(k == 0), stop=(k == 3))
                if pj_state["eng"] % 2 == 0:
                    nc.scalar.copy(out=ob[:, v0:v0 + vw], in_=pst[:, :vw])
                else:
                    nc.vector.tensor_copy(out=ob[:, v0:v0 + vw],
                                          in_=pst[:, :vw])
                pj_state["eng"] += 1
            nc.sync.dma_start(io["out"][mth * P:(mth + 1) * P, c0:c0 + cw],
                              ob[:, :cw])

    hT_f, hT_b = h0T_f, h0T_b
    m = Bc
    for t in range(T):
        av = (t - 16) // 16
        ppr = pq.tile([16, 512], f32, tag="ppr")
        ppz = pq.tile([16, 512], f32, tag="ppz")
        ppn = pq.tile([16, 512], f32, tag="ppn")
        ppx = pq.tile([16, 512], f32, tag="ppx")
        # u-part first (depends only on h): nh, r, z
        nc.tensor.matmul(ppn[:m, :], onesb[:1, :m], bnd_sb[:1, :],
                         start=True, stop=False)
        for k in range(4):
            nc.tensor.matmul(ppn[:m, :], hT_b[:, k, :m],
                             ud_sb[:, k, 1024:1536],
                             start=False, stop=(k == 3))
        for k in range(4):
            nc.tensor.matmul(ppr[:m, :], hT_b[:, k, :m], ud_sb[:, k, 0:512],
                             start=(k == 0), stop=(t == 0 and k == 3))
        for k in range(4):
            nc.tensor.matmul(ppz[:m, :], hT_b[:, k, :m],
                             ud_sb[:, k, 512:1024],
                             start=(k == 0), stop=(t == 0 and k == 3))
        # wh-part (depends on ht from previous step): r, z, nx
        if t > 0:
            htprev = htall[:, :, (t - 1) * m:t * m]
            for k in range(4):
                nc.tensor.matmul(ppr[:m, :], htprev[:, k, :],
                                 whd_sb[:, k, 0:512],
                                 start=False, stop=(k == 3))
            for k in range(4):
                nc.tensor.matmul(ppz[:m, :], htprev[:, k, :],
                                 whd_sb[:, k, 512:1024],
                                 start=False, stop=(k == 3))
            for k in range(4):
                nc.tensor.matmul(ppx[:m, :], htprev[:, k, :],
                                 whd_sb[:, k, 1024:1536],
                                 start=(k == 0), stop=(k == 3))
        # crossings
        cr = wpool.tile([16, H], f32, tag="cr", bufs=1)
        nc.scalar.copy(out=cr[:m, :], in_=ppr[:m, :])
        cz = wpool.tile([16, H], f32, tag="cz", bufs=1)
        nc.scalar.copy(out=cz[:m, :], in_=ppz[:m, :])
        cn = wpool.tile([16, H], f32, tag="cn", bufs=1)
        nc.vector.tensor_copy(out=cn[:m, :], in_=ppn[:m, :])
        if t > 0:
            cx = wpool.tile([16, H], f32, tag="cx", bufs=1)
            nc.vector.tensor_copy(out=cx[:m, :], in_=ppx[:m, :])
        emit_proj(1 if t >= 16 else 0, av)
        tp = pt.tile([P, 16, 16], f32, tag="tp")
        for k in range(4):
            nc.tensor.transpose(tp[:, 0 + k, :m], cr[:m, k * P:(k + 1) * P],
                                ident[:m, :m])
        for k in range(4):
            nc.tensor.transpose(tp[:, 4 + k, :m], cz[:m, k * P:(k + 1) * P],
                                ident[:m, :m])
        for k in range(4):
            nc.tensor.transpose(tp[:, 8 + k, :m], cn[:m, k * P:(k + 1) * P],
                                ident[:m, :m])
        if t > 0:
            for k in range(4):
                nc.tensor.transpose(tp[:, 12 + k, :m],
                                    cx[:m, k * P:(k + 1) * P], ident[:m, :m])
        emit_proj(1 if t >= 16 else 0, av)
        hT_f, hT_b = gru_tail(m, tp, xpxT, t, hT_f, t > 0)
        hb_new = hT_b
        # ---- attention; amask folded into the scores psum via matmul
        pst_sc = ps.tile([P, 512], f32, tag="sc")
        for k in range(4):
            nc.tensor.matmul(pst_sc[:m, :NSEQ], hb_new[:, k, :m], gT[:, k, :],
                             start=(k == 0), stop=False)
        nc.tensor.matmul(pst_sc[:m, :NSEQ], identb[:m, :m], amask_sb[:],
                         start=False, stop=True)
        emit_proj(1 if t >= 16 else 0, av)
        alpha = wpool.tile([Bc, NSEQ], f32, tag="alpha")
        sexp = wpool.tile([Bc, 1], f32, tag="sexp")
        nc.scalar.activation(alpha[:], pst_sc[:m, :NSEQ], AF.Exp,
                             accum_out=sexp[:])
        rs = wpool.tile([Bc, 1], f32, tag="rs")
        nc.vector.reciprocal(rs[:], sexp[:])
        alpha2 = wpool.tile([Bc, NSEQ], f32, tag="alpha2")
        nc.vector.tensor_scalar_mul(alpha2[:], alpha[:], rs[:])
        tpa = pt.tile([P, 16, 16], f32, tag="tp")
        for j in range(3):
            nc.tensor.transpose(tpa[:, j, :m], alpha2[:, j * P:(j + 1) * P],
                                ident[:m, :m])
        aT = wpool.tile([P, 3, Bc], bf16, tag="aT")
        nc.vector.tensor_copy(out=aT[:], in_=tpa[:, 0:3, :m])
        htp = ps.tile([P, 512], f32, tag="sc")
        for j in range(3):
            nc.tensor.matmul(htp[:m, :H], aT[:, j, :], pf[:, j, :],
                             start=(j == 0), stop=False)
        for k in range(4):
            nc.tensor.matmul(htp[:m, :H], hb_new[:, k, :m], wch_sb[:, k, :],
                             start=False, stop=(k == 3))
        htv = wpool.tile([Bc, H], f32, tag="htv")
        nc.scalar.activation(htv[:], htp[:m, :H], AF.Tanh)
        tph = pt.tile([P, 16, 16], f32, tag="tp")
        for k in range(4):
            nc.tensor.transpose(tph[:, k, :m], htv[:, k * P:(k + 1) * P],
                                ident[:m, :m])
        nc.scalar.copy(out=htall[:, :, t * m:(t + 1) * m],
                       in_=tph[:, 0:4, :m])

    # drain remaining projection jobs
    emit_proj(len(proj_jobs), 2)

    if io["dbg"]:
        nc.sync.dma_start(io["dbg_l0T"][:], l0T[:])
        nc.sync.dma_start(io["dbg_hencT"][:], hencT[:])
        nc.sync.dma_start(io["dbg_htall"][:], htall[:])

    for cm in reversed(cms):
        cm.__exit__(None, None, None)


# ---------------------------------------------------------------- host side
_PROGRAM = None


def _get_program():
    global _PROGRAM
    if _PROGRAM is None:
        _install_profile_hook()
        _PROGRAM = build_program(dbg=os.environ.get("KDBG", "0") == "1")
    return _PROGRAM


def _prep_shared(inputs):
    import ml_dtypes
    f = np.float32
    bf = ml_dtypes.bfloat16

    def b(x):
        return np.ascontiguousarray(np.asarray(x, f).astype(bf))

    def wall(wih, bih):
        # full Wih.T with r/z columns prescaled by 0.5; bias transposed
        wt = np.asarray(wih, f).T.copy()           # (in, 3H)
        bb = np.asarray(bih, f).copy()             # (3H,)
        wt[:, 0:H2] *= 0.5
        bb2 = bb.copy()
        bb2[0:H2] *= 0.5
        bT = np.ascontiguousarray(bb2.reshape(12, P).T)   # (128, 12) f32
        return b(wt), bT

    g = {}
    g["w0a"], g["b0T"] = wall(inputs["enc0_Wih"], inputs["enc0_bih"])
    g["u0"] = b(np.concatenate([np.asarray(inputs["enc0_Ur"], f).T,
                                np.asarray(inputs["enc0_Uz"], f).T,
                                np.asarray(inputs["enc0_Un"], f).T], axis=1))
    g["bn0"] = b(np.asarray(inputs["enc0_bn"], f)[None, :])
    g["w1a"], g["b1T"] = wall(inputs["enc1_Wih"], inputs["enc1_bih"])
    g["u1"] = b(np.concatenate([np.asarray(inputs["enc1_Ur"], f).T,
                                np.asarray(inputs["enc1_Uz"], f).T,
                                np.asarray(inputs["enc1_Un"], f).T], axis=1))
    g["bn1"] = b(np.asarray(inputs["enc1_bn"], f)[None, :])
    g["fct"] = b(np.asarray(inputs["fc_init_w"], f).T)
    g["fcb"] = b(np.asarray(inputs["fc_init_b"], f)[None, :])
    scale = np.float32(1.0) / np.sqrt(np.float32(H2))
    g["was"] = b(np.asarray(inputs["Wa"], f) * scale)
    acw = np.asarray(inputs["attn_combine_w"], f)
    g["wch"] = b(acw[:, :H].T)
    g["wcc"] = b(acw[:, H:].T)
    dwih = np.asarray(inputs["dec_Wih"], f)
    g["wxa"], g["bdT"] = wall(dwih[:, :H], inputs["dec_bih"])
    g["whd"] = b(dwih[:, H:].T)
    g["ud"] = b(np.concatenate([np.asarray(inputs["dec_Ur"], f).T,
                                np.asarray(inputs["dec_Uz"], f).T,
                                np.asarray(inputs["dec_Un"], f).T], axis=1))
    g["bnd"] = b(np.asarray(inputs["dec_bn"], f)[None, :])
    g["owt"] = b(np.asarray(inputs["out_w"], f).T)
    return g


def _prep_core(inputs, c):
    import ml_dtypes
    bf = ml_dtypes.bfloat16
    src = np.asarray(inputs["src"])
    tgt = np.asarray(inputs["tgt"])
    emb = np.asarray(inputs["emb"], np.float32)
    si = src[:, c * Bc:(c + 1) * Bc].astype(np.int64)      # (48, 8)
    ti = tgt[:, c * Bc:(c + 1) * Bc].astype(np.int64)
    idx_enc = np.empty((T, 2, Bc), np.int64)
    idx_enc[:, 0, :] = si
    idx_enc[:, 1, :] = si[::-1]
    xeT_in = np.ascontiguousarray(emb[idx_enc.reshape(NSCAN)].T.astype(bf))
    xdT_in = np.ascontiguousarray(emb[ti.reshape(NSEQ)].T.astype(bf))
    mk = np.full((Bc, T, Bc), NEG, np.float32)
    for bb in range(Bc):
        mk[bb, :, bb] = np.where(si[:, bb] != 0, np.float32(0.0),
                                 np.float32(NEG))
    return {"xeT_in": xeT_in,
            "xdT_in": xdT_in,
            "amask": np.ascontiguousarray(mk.reshape(Bc, NSEQ).astype(bf))}


def kernel(**inputs):
    nc = _get_program()
    shared = _prep_shared(inputs)
    in_maps = []
    for c in range(NCORES):
        im = dict(shared)
        im.update(_prep_core(inputs, c))
        in_maps.append(im)
    res = run_bass_kernel_spmd(nc, in_maps, core_ids=list(range(NCORES)))
    logits = np.empty((T, B, V), np.float32)
    for c in range(NCORES):
        logits[:, c * Bc:(c + 1) * Bc, :] = \
            res.results[c]["out"].astype(np.float32).reshape(T, Bc, V)
    return logits
